# revision 13
# baseline (speedup 1.0000x reference)
"""ComplexityAwareAttention Trainium2 Bass kernel (v3 schedule).

Sharding: 8 cores = 2 batches x 4 head-groups (3 heads each). Each core
computes q/k/v projections for its 3 heads, masked-key-gathered attention
(keys with attention_mask==0 are removed on host), and a partial output
projection (2048, 768). Host sums the 4 partials per batch and adds the
fused output bias (bo + Wo @ bv).

v3 schedule: paced by ScalarE's exp stream (48 tiles of [128,1024]).
Lead-in is DMA-critical-path minimized (4 parallel queues, kproj starts
on a 128-key slice to warm the PE early). All other projection work is
spread as <=3-matmul half-units, one per attention step, so the PE feeds
the exp stream without burst gaps. Softmax normalize: copy denom row ->
reciprocal_approx_fast -> Pool partition_broadcast -> full-width DVE
multiply (512-col chunks; wider PSUM APs cross banks and misbehave in
the custom DVE op). Tail: attention PSUM pools are closed and a
4-buffer tail pool ping-pongs the half-1 output projection with casts
split across ScalarE and DVE.

PSUM budget (8 banks): sT double-buffer (4) + oacc (2) + projection
ping-pong (2); tail reuses 4 banks after the attention pools close.
"""

import math
import os
from contextlib import ExitStack

import numpy as np

import concourse.bass as bass
from concourse import bacc
import concourse.mybir as mybir
import concourse.tile as tile
from concourse.bass import ds, ts
from concourse.bass_utils import run_bass_kernel_spmd

F32 = mybir.dt.float32
F16 = mybir.dt.float16
AFT = mybir.ActivationFunctionType

B = 2
S = 2048
D = 768
H = 12
HD = 64
NH = 3  # heads per core
KT_D = D // 128  # 6 contraction tiles over d_model
SCORE_OFF = 12.5  # subtracted inside exp so et fits fp16 (scores reach ~22)

LAST_EXEC_TIME_NS = None
LAST_RESULTS = None


def build_nc(nk_t):
    n_k = nk_t * 128
    nkc = (n_k + 511) // 512  # xkT 512-col chunks
    nkp = nkc * 512  # padded key columns
    nc = bacc.Bacc(None, target_bir_lowering=False)

    # xT packed as (128, half, qc, kt, 512) so each (half, qc) projection
    # unit's DMA slice is contiguous per partition.
    d_xT = nc.dram_tensor("xT", (128, 2, 2, KT_D, 512), F16, kind="ExternalInput")
    d_xkT = nc.dram_tensor("xkT", (128, nkc, KT_D, 512), F16, kind="ExternalInput")
    d_wq = nc.dram_tensor("wq", (128, KT_D, 192), F16, kind="ExternalInput")
    d_wk = nc.dram_tensor("wk", (128, KT_D, 192), F16, kind="ExternalInput")
    d_wv = nc.dram_tensor("wv", (128, KT_D, 192), F16, kind="ExternalInput")
    d_wo = nc.dram_tensor("wo", (128, 2, D), F16, kind="ExternalInput")
    d_bq = nc.dram_tensor("bq", (128, 2), F32, kind="ExternalInput")
    d_bk = nc.dram_tensor("bk", (128, 2), F32, kind="ExternalInput")
    d_vcol = nc.dram_tensor("vcol", (128, nk_t), F16, kind="ExternalInput")
    d_out = nc.dram_tensor("out", (128, 16, D), F16, kind="ExternalOutput")

    with ExitStack() as ctx:
        tc = ctx.enter_context(tile.TileContext(nc))
        singles = ctx.enter_context(tc.tile_pool(name="singles", bufs=1))
        etp = ctx.enter_context(tc.tile_pool(name="etp", bufs=6))
        rowp = ctx.enter_context(tc.tile_pool(name="rowp", bufs=2))
        bcp = ctx.enter_context(tc.tile_pool(name="bcp", bufs=2))
        ogp = ctx.enter_context(tc.tile_pool(name="ogp", bufs=4))

        # Pull the Exp activation table load off the critical path.
        dummy = singles.tile([1, 2], F32)
        nc.vector.memset(dummy, 0.0)
        nc.scalar.activation(dummy, dummy, AFT.Exp)
        # per-partition exp bias (score offset; cancels in softmax)
        sb_soff = singles.tile([128, 1], F32)
        nc.vector.memset(sb_soff, -SCORE_OFF)

        sb_xT = singles.tile([128, 2, 2, KT_D, 512], F16)
        sb_xkT = singles.tile([128, nkc, KT_D, 512], F16)
        sb_wq = singles.tile([128, KT_D, 192], F16)
        sb_wk = singles.tile([128, KT_D, 192], F16)
        sb_wv = singles.tile([128, KT_D, 192], F16)
        sb_wo = singles.tile([128, 2, D], F16)
        sb_bq = singles.tile([128, 2], F32)
        sb_bk = singles.tile([128, 2], F32)
        sb_v = singles.tile([128, NH, nk_t, 65], F16)
        sb_qT = singles.tile([128, 2, S], F16)
        sb_kT = singles.tile([128, 2, nkp], F16)
        sb_onT = singles.tile([128, 2, S], F16)

        # ---- DMA: minimize the critical prefix (kproj: wk + xkT c0 key
        # slice a; qproj: wq + xT(0,qc)) by spreading over 4 queues.
        nc.scalar.dma_start(out=sb_xkT[:, 0, :, 0:128], in_=d_xkT[:, 0, :, 0:128])
        nc.scalar.dma_start(out=sb_xkT[:, 0, :, 128:512], in_=d_xkT[:, 0, :, 128:512])
        for c in range(1, nkc):
            nc.scalar.dma_start(out=sb_xkT[:, c], in_=d_xkT[:, c])
        nc.scalar.dma_start(out=sb_wo, in_=d_wo[:, :, :])
        nc.sync.dma_start(out=sb_xT[:, 0, 0], in_=d_xT[:, 0, 0])
        nc.sync.dma_start(out=sb_xT[:, 0, 1], in_=d_xT[:, 0, 1])
        nc.sync.dma_start(out=sb_xT[:, 1, 0], in_=d_xT[:, 1, 0])
        nc.sync.dma_start(out=sb_xT[:, 1, 1], in_=d_xT[:, 1, 1])
        nc.gpsimd.dma_start(out=sb_wq, in_=d_wq[:, :, :])
        nc.gpsimd.dma_start(out=sb_wk, in_=d_wk[:, :, :])
        nc.gpsimd.dma_start(out=sb_wv, in_=d_wv[:, :, :])
        nc.gpsimd.dma_start(out=sb_bq, in_=d_bq[:, :])
        nc.gpsimd.dma_start(out=sb_bk, in_=d_bk[:, :])
        for h in range(NH):
            nc.gpsimd.dma_start(out=sb_v[:, h, :, 64:65], in_=d_vcol[:, :])

        stack2 = ctx.enter_context(ExitStack())
        stp = stack2.enter_context(tc.tile_pool(name="stp", bufs=2, space="PSUM"))
        oap = stack2.enter_context(tc.tile_pool(name="oap", bufs=1, space="PSUM"))
        pjp = stack2.enter_context(tc.tile_pool(name="pjp", bufs=2, space="PSUM"))

        # ---- projection / drain unit helpers ----
        def kproj_cols(m, c, o0, o1, ps, first, last):
            rows = 128 if m == 0 else 64
            msl = ds(m * 128, rows)
            for kt in range(KT_D):
                nc.tensor.matmul(
                    ps[:rows, o0 - o0 : o1 - o0],
                    sb_wk[:, kt, msl],
                    sb_xkT[:, c, kt, o0:o1],
                    start=(kt == 0),
                    stop=(kt == KT_D - 1),
                )
            nc.vector.tensor_scalar_add(
                out=sb_kT[:rows, m, ds(c * 512 + o0, o1 - o0)],
                in0=ps[:rows, 0 : o1 - o0],
                scalar1=sb_bk[:rows, m : m + 1],
            )

        def qproj_half(half, m, qc, ps, lo):
            # 3 contraction tiles; lo selects kt 0-2 (start) or 3-5 (stop)
            rows = 128 if m == 0 else 64
            msl = ds(m * 128, rows)
            for kt in range(3 * lo, 3 * lo + 3):
                nc.tensor.matmul(
                    ps[:rows, :],
                    sb_wq[:, kt, msl],
                    sb_xT[:, half, qc, kt, :],
                    start=(kt == 0),
                    stop=(kt == KT_D - 1),
                )
            if lo == 1:
                nc.vector.tensor_scalar_add(
                    out=sb_qT[:rows, m, ds(half * 1024 + qc * 512, 512)],
                    in0=ps[:rows, :],
                    scalar1=sb_bq[:rows, m : m + 1],
                )

        def kproj_half(m, c, ps, lo):
            rows = 128 if m == 0 else 64
            msl = ds(m * 128, rows)
            for kt in range(3 * lo, 3 * lo + 3):
                nc.tensor.matmul(
                    ps[:rows, :],
                    sb_wk[:, kt, msl],
                    sb_xkT[:, c, kt, :],
                    start=(kt == 0),
                    stop=(kt == KT_D - 1),
                )
            if lo == 1:
                nc.vector.tensor_scalar_add(
                    out=sb_kT[:rows, m, ds(c * 512, 512)],
                    in0=ps[:rows, :],
                    scalar1=sb_bk[:rows, m : m + 1],
                )

        def vproj(kt2):
            c, off = (kt2 * 128) // 512, (kt2 * 128) % 512
            ps = pjp.tile([128, 512], F32, tag="ps")
            for kt in range(KT_D):
                nc.tensor.matmul(
                    ps[:, 0:192],
                    sb_xkT[:, c, kt, ds(off, 128)],
                    sb_wv[:, kt, :],
                    start=(kt == 0),
                    stop=(kt == KT_D - 1),
                )
            nc.vector.tensor_copy(
                out=sb_v[:, :, kt2, 0:64],
                in_=ps[:, 0:192].rearrange("p (h d) -> p h d", h=NH),
            )

        def oproj_chunk(qt, og, j, eoff, ech, pool, cast):
            ps = pool.tile([128, 512], F32, tag="ps")
            nc.tensor.matmul(
                ps[:, :ech],
                sb_onT[:, 0, ts(qt, 128)],
                sb_wo[:, 0, ds(eoff, ech)],
                start=True,
                stop=False,
            )
            nc.tensor.matmul(
                ps[:, :ech],
                sb_onT[0:64, 1, ts(qt, 128)],
                sb_wo[0:64, 1, ds(eoff, ech)],
                start=False,
                stop=True,
            )
            if cast == "scalar":
                nc.scalar.copy(out=og[:, j, ds(eoff, ech)], in_=ps[:, :ech])
            else:
                nc.vector.tensor_copy(out=og[:, j, ds(eoff, ech)], in_=ps[:, :ech])

        # ---- attention step helpers ----
        def head_rows(head):
            qrow = 64 if head == 1 else 0
            slot = 1 if head == 2 else 0
            return qrow, slot

        def qk(head, half, kt2, sT):
            qrow, slot = head_rows(head)
            for qc in range(2):
                nc.tensor.matmul(
                    sT[:, ts(qc, 512)],
                    sb_kT[ds(qrow, 64), slot, ts(kt2, 128)],
                    sb_qT[ds(qrow, 64), slot, ds(half * 1024 + qc * 512, 512)],
                    start=True,
                    stop=True,
                )

        def pv(head, kt2, et, oacc):
            for qc in range(2):
                nc.tensor.matmul(
                    oacc[:, ts(qc, 512)],
                    sb_v[:, head, kt2, :],
                    et[:, ts(qc, 512)],
                    start=(kt2 == 0),
                    stop=(kt2 == nk_t - 1),
                )

        def norm(head, half, oacc):
            qrow, slot = head_rows(head)
            for ch in range(2):
                csl = ds(ch * 512, 512)
                drow = rowp.tile([1, 512], F32, tag="drow")
                nc.vector.tensor_copy(out=drow, in_=oacc[64:65, csl])
                rrow = rowp.tile([1, 512], F32, tag="rrow")
                nc.vector.reciprocal_approx_fast(out=rrow, in_=drow)
                rb = bcp.tile([64, 512], F32, tag="rb")
                nc.gpsimd.partition_broadcast(rb, rrow)
                nc.vector.tensor_mul(
                    out=sb_onT[ds(qrow, 64), slot, ds(half * 1024 + ch * 512, 512)],
                    in0=oacc[0:64, csl],
                    in1=rb,
                )

        # ---- deferred unit schedule: at most one half-unit per step ----
        og_h0 = {}

        def oproj_h0(qt):
            if qt % 2 == 0:
                og_h0[qt // 2] = ogp.tile([128, 2, D], F16, tag="og", name="og")
            og = og_h0[qt // 2]
            j = qt % 2
            oproj_chunk(qt, og, j, 0, 512, pjp, "vector")
            oproj_chunk(qt, og, j, 512, 256, pjp, "vector")
            if qt % 2 == 1:
                nc.sync.dma_start(out=d_out[:, ds(qt - 1, 2), :], in_=og)

        units = {}

        def add_unit(step, th):
            units.setdefault(step, []).append(th)

        kp_ps = {}

        def kproj_u(m, c, lo):
            key = (m, c)
            if lo == 0:
                kp_ps[key] = pjp.tile([128, 512], F32, tag="ps", name="ps")
            kproj_half(m, c, kp_ps[key], lo)

        qp_ps = {}

        def qproj_u(half, m, qc, lo):
            key = (half, m, qc)
            if lo == 0:
                qp_ps[key] = pjp.tile([128, 512], F32, tag="ps", name="ps")
            qproj_half(half, m, qc, qp_ps[key], lo)

        # vproj(kt2) feeds PV(A, h0, kt2) one step later; bunch 0-3 into
        # steps 0-1 to free steps 2-3 for the kproj slot0 chunk-1 halves
        # (QK(A, kt2=4) needs kT cols 512+ at step 4).
        add_unit(0, lambda: vproj(0))
        add_unit(0, lambda: vproj(1))
        add_unit(1, lambda: vproj(2))
        add_unit(1, lambda: vproj(3))
        for kt2 in range(4, nk_t):
            add_unit(kt2, lambda k=kt2: vproj(k))
        s = 2
        for c in range(1, nkc):
            add_unit(s, lambda cc=c: kproj_u(0, cc, 0))
            add_unit(s + 1, lambda cc=c: kproj_u(0, cc, 1))
            s += 2
        # kproj m=1 (head C) as half-units; needed by C-h0 at 2*nk_t
        base = nk_t
        for c in range(nkc):
            add_unit(base + 2 * c, lambda cc=c: kproj_u(1, cc, 0))
            add_unit(base + 2 * c + 1, lambda cc=c: kproj_u(1, cc, 1))
        qb = base + 2 * nkc
        add_unit(qb + 0, lambda: qproj_u(0, 1, 0, 0))
        add_unit(qb + 1, lambda: qproj_u(0, 1, 0, 1))
        add_unit(qb + 2, lambda: qproj_u(0, 1, 1, 0))
        add_unit(qb + 3, lambda: qproj_u(0, 1, 1, 1))
        # during C-h0 (steps 2*nk_t..): q projections for half 1
        b2 = 2 * nk_t
        add_unit(b2 + 0, lambda: qproj_u(1, 0, 0, 0))
        add_unit(b2 + 1, lambda: qproj_u(1, 0, 0, 1))
        add_unit(b2 + 2, lambda: qproj_u(1, 0, 1, 0))
        add_unit(b2 + 3, lambda: qproj_u(1, 0, 1, 1))
        add_unit(b2 + 4, lambda: qproj_u(1, 1, 0, 0))
        add_unit(b2 + 5, lambda: qproj_u(1, 1, 0, 1))
        add_unit(b2 + 6, lambda: qproj_u(1, 1, 1, 0))
        add_unit(b2 + 7, lambda: qproj_u(1, 1, 1, 1))
        # oproj for half 0 (q tiles 0-7), one tile every other step during
        # half-1 attention
        b3 = 3 * nk_t
        for i in range(8):
            add_unit(b3 + 1 + 2 * i, lambda q=i: oproj_h0(q))

        # ---- lead-in: minimum to start (A, h0) ----
        ps0 = pjp.tile([128, 512], F32, tag="ps")
        kproj_cols(0, 0, 0, 128, ps0, True, True)
        ps1 = pjp.tile([128, 512], F32, tag="ps")
        kproj_cols(0, 0, 128, 512, ps1, True, True)
        qproj_u(0, 0, 0, 0)
        qproj_u(0, 0, 0, 1)
        qproj_u(0, 0, 1, 0)
        qproj_u(0, 0, 1, 1)

        # ---- 48-step attention pipeline ----
        order = [(0, 0), (0, 1), (0, 2), (1, 0), (1, 1), (1, 2)]
        prev = None
        step = 0
        for half, head in order:
            oacc = oap.tile([65, 1024], F32, tag="oacc")
            for kt2 in range(nk_t):
                sT = stp.tile([128, 1024], F32, tag="sT")
                qk(head, half, kt2, sT)
                et = etp.tile([128, 1024], F16, tag="et")
                nc.scalar.activation(et, sT, AFT.Exp, bias=sb_soff[:, 0:1])
                if prev is not None:
                    ph, phalf, pkt2, pet, poacc = prev
                    pv(ph, pkt2, pet, poacc)
                    if pkt2 == nk_t - 1:
                        norm(ph, phalf, poacc)
                for th in units.pop(step, []):
                    th()
                prev = (head, half, kt2, et, oacc)
                step += 1
        ph, phalf, pkt2, pet, poacc = prev
        pv(ph, pkt2, pet, poacc)
        norm(ph, phalf, poacc)
        for s in sorted(units):
            for th in units.pop(s):
                th()

        # close attention PSUM pools; tail gets its own 4-buffer pool
        stack2.close()
        tailp = ctx.enter_context(tc.tile_pool(name="tailp", bufs=4, space="PSUM"))

        # ---- tail: oproj for half 1 (q tiles 8-15) ----
        for i in range(4):
            og = ogp.tile([128, 2, D], F16, tag="og")
            for j in range(2):
                qt = 8 + 2 * i + j
                oproj_chunk(qt, og, j, 0, 512, tailp, "scalar")
                oproj_chunk(qt, og, j, 512, 256, tailp, "vector")
            eng = nc.sync if i % 2 == 0 else nc.scalar
            eng.dma_start(out=d_out[:, ds(8 + 2 * i, 2), :], in_=og)

    nc.compile()
    return nc


def kernel(
    hidden_states,
    complexity_scores,
    attention_mask,
    Wq,
    bq,
    Wk,
    bk,
    Wv,
    bv,
    Wo,
    bo,
    emb_table,
    comp_scaling,
):
    global LAST_EXEC_TIME_NS, LAST_RESULTS
    hs = np.asarray(hidden_states, np.float32)
    cs = np.asarray(complexity_scores).astype(np.int64)
    am = np.asarray(attention_mask)
    Wq = np.asarray(Wq, np.float32)
    bq = np.asarray(bq, np.float32)
    Wk = np.asarray(Wk, np.float32)
    bk = np.asarray(bk, np.float32)
    Wv = np.asarray(Wv, np.float32)
    bv = np.asarray(bv, np.float32)
    Wo = np.asarray(Wo, np.float32)
    bo = np.asarray(bo, np.float32)
    emb_table = np.asarray(emb_table, np.float32)
    comp_scaling = np.asarray(comp_scaling, np.float32)

    # per-head score scale (identical across batch: mean over batch of embs)
    embs = emb_table[cs]  # (B, H)
    scal = comp_scaling * embs.mean(axis=0)  # (H,)
    c = (scal / math.sqrt(HD)).astype(np.float32)

    # gather unmasked keys per batch; pad to a common multiple of 128
    idx = [np.nonzero(am[b] != 0)[0] for b in range(B)]
    n_max = max(1, max(len(i) for i in idx))
    nk_t = max(2, (n_max + 127) // 128)
    n_k = nk_t * 128
    nkc = (n_k + 511) // 512
    nkp = nkc * 512

    xT = []
    xkT = []
    vcol = []
    for b in range(B):
        t = hs[b].T.astype(np.float16)  # (768, 2048)
        # (128, half, qc, kt, 512)
        xT.append(
            np.ascontiguousarray(
                t.reshape(KT_D, 128, 2, 2, 512).transpose(1, 2, 3, 0, 4)
            )
        )
        tk = np.zeros((D, nkp), np.float16)
        tk[:, : len(idx[b])] = hs[b][idx[b]].T
        xkT.append(
            np.ascontiguousarray(
                tk.reshape(KT_D, 128, nkc, 512).transpose(1, 2, 0, 3)
            )
        )
        v = np.zeros((nk_t * 128,), np.float16)
        v[: len(idx[b])] = 1.0
        vcol.append(np.ascontiguousarray(v.reshape(nk_t, 128).T))

    WqT = Wq.T  # (d_in, e_out)
    WkT = Wk.T
    WvT = Wv.T
    WoT = np.ascontiguousarray(Wo.T)  # rows = attended feature d

    def pack_w(w192):  # (768, 192) -> (128, KT_D, 192)
        return np.ascontiguousarray(
            w192.astype(np.float16).reshape(KT_D, 128, 192).transpose(1, 0, 2)
        )

    def pack_bias(vec):  # (192,) -> (128, 2)
        out = np.zeros((128, 2), np.float32)
        out[:, 0] = vec[:128]
        out[:64, 1] = vec[128:]
        return out

    in_maps = []
    for core in range(8):
        b = core // 4
        heads = [3 * (core % 4) + j for j in range(NH)]
        cols = np.concatenate([np.arange(h * HD, (h + 1) * HD) for h in heads])
        cscale = np.repeat(c[heads], HD)  # (192,)
        wq_c = pack_w(WqT[:, cols] * cscale[None, :])
        bq_c = bq[cols] * cscale
        wk_c = pack_w(WkT[:, cols])
        bk_c = bk[cols]
        wv_c = pack_w(WvT[:, cols])
        wo_c = np.zeros((128, 2, D), np.float16)
        wo_c[:, 0, :] = WoT[cols[:128], :]
        wo_c[:64, 1, :] = WoT[cols[128:], :]
        in_maps.append(
            {
                "xT": xT[b],
                "xkT": xkT[b],
                "wq": wq_c,
                "wk": wk_c,
                "wv": wv_c,
                "wo": np.ascontiguousarray(wo_c),
                "bq": pack_bias(bq_c),
                "bk": pack_bias(bk_c),
                "vcol": vcol[b],
            }
        )

    nc = build_nc(nk_t)
    trace = os.environ.get("KERNEL_TRACE", "0") == "1"
    res = run_bass_kernel_spmd(nc, in_maps, core_ids=list(range(8)), trace=trace)
    LAST_EXEC_TIME_NS = res.exec_time_ns
    LAST_RESULTS = res

    bo_eff = (bo + Wo @ bv).astype(np.float64)
    out = np.empty((B, S, D), np.float32)
    for b in range(B):
        acc = np.zeros((S, D), np.float64)
        for g in range(4):
            p = res.results[4 * b + g]["out"]  # (128, 16, D) fp16
            acc += p.astype(np.float64).transpose(1, 0, 2).reshape(S, D)
        out[b] = (acc + bo_eff[None, :]).astype(np.float32)
    return out


# revision 21
# speedup vs baseline: 1.0842x; 1.0842x over previous
"""ComplexityAwareAttention Trainium2 Bass kernel (v3 schedule).

Sharding: 8 cores = 2 batches x 4 head-groups (3 heads each). Each core
computes q/k/v projections for its 3 heads, masked-key-gathered attention
(keys with attention_mask==0 are removed on host), and a partial output
projection (2048, 768). Host sums the 4 partials per batch and adds the
fused output bias (bo + Wo @ bv).

v3 schedule: paced by ScalarE's exp stream (48 tiles of [128,1024]).
Lead-in is DMA-critical-path minimized (4 parallel queues, kproj starts
on a 128-key slice to warm the PE early). All other projection work is
spread as <=3-matmul half-units, one per attention step, so the PE feeds
the exp stream without burst gaps. Softmax normalize: copy denom row ->
reciprocal_approx_fast -> Pool partition_broadcast -> full-width DVE
multiply (512-col chunks; wider PSUM APs cross banks and misbehave in
the custom DVE op). Tail: attention PSUM pools are closed and a
4-buffer tail pool ping-pongs the half-1 output projection with casts
split across ScalarE and DVE.

PSUM budget (8 banks): sT double-buffer (4) + oacc (2) + projection
ping-pong (2); tail reuses 4 banks after the attention pools close.
"""

import math
import os
from contextlib import ExitStack

import numpy as np

import concourse.bass as bass
from concourse import bacc
import concourse.mybir as mybir
import concourse.tile as tile
from concourse.bass import ds, ts
from concourse.bass_utils import run_bass_kernel_spmd

F32 = mybir.dt.float32
F16 = mybir.dt.float16
AFT = mybir.ActivationFunctionType

B = 2
S = 2048
D = 768
H = 12
HD = 64
NH = 3  # heads per core
KT_D = D // 128  # 6 contraction tiles over d_model
SCORE_OFF = 12.5  # subtracted inside exp so et fits fp16 (scores reach ~22)

LAST_EXEC_TIME_NS = None
LAST_RESULTS = None


def build_nc(nk_t):
    n_k = nk_t * 128
    nkc = (n_k + 511) // 512  # xkT 512-col chunks
    nkp = nkc * 512  # padded key columns
    nc = bacc.Bacc(None, target_bir_lowering=False)

    # xT packed as (128, half, qc, kt, 512) so each (half, qc) projection
    # unit's DMA slice is contiguous per partition.
    d_xT = nc.dram_tensor("xT", (128, 2, 2, KT_D, 512), F16, kind="ExternalInput")
    # key-tile-major so every DMA prefix is contiguous per partition
    d_xkT = nc.dram_tensor("xkT", (128, nk_t, KT_D, 128), F16, kind="ExternalInput")
    d_wq = nc.dram_tensor("wq", (128, KT_D, 192), F16, kind="ExternalInput")
    d_wk = nc.dram_tensor("wk", (128, KT_D, 192), F16, kind="ExternalInput")
    d_wv = nc.dram_tensor("wv", (128, KT_D, 192), F16, kind="ExternalInput")
    d_wo = nc.dram_tensor("wo", (128, 2, D), F16, kind="ExternalInput")
    d_bq = nc.dram_tensor("bq", (128, 2), F32, kind="ExternalInput")
    d_bk = nc.dram_tensor("bk", (128, 2), F32, kind="ExternalInput")
    d_vcol = nc.dram_tensor("vcol", (128, nk_t), F16, kind="ExternalInput")
    d_out = nc.dram_tensor("out", (128, 16, D), F16, kind="ExternalOutput")

    with ExitStack() as ctx:
        tc = ctx.enter_context(tile.TileContext(nc))
        singles = ctx.enter_context(tc.tile_pool(name="singles", bufs=1))
        etp = ctx.enter_context(tc.tile_pool(name="etp", bufs=6))
        rowp = ctx.enter_context(tc.tile_pool(name="rowp", bufs=2))
        bcp = ctx.enter_context(tc.tile_pool(name="bcp", bufs=2))
        ogp = ctx.enter_context(tc.tile_pool(name="ogp", bufs=4))

        # Pull the Exp activation table load off the critical path.
        dummy = singles.tile([1, 2], F32)
        nc.vector.memset(dummy, 0.0)
        nc.scalar.activation(dummy, dummy, AFT.Exp)
        # per-partition exp bias (score offset; cancels in softmax)
        sb_soff = singles.tile([128, 1], F32)
        nc.vector.memset(sb_soff, -SCORE_OFF)

        sb_xT = singles.tile([128, 2, 2, KT_D, 512], F16)
        sb_xkT = singles.tile([128, nk_t, KT_D, 128], F16)
        sb_wq = singles.tile([128, KT_D, 192], F16)
        sb_wk = singles.tile([128, KT_D, 192], F16)
        sb_wv = singles.tile([128, KT_D, 192], F16)
        sb_wo = singles.tile([128, 2, D], F16)
        sb_bq = singles.tile([128, 2], F32)
        sb_bk = singles.tile([128, 2], F32)
        sb_v = singles.tile([128, NH, nk_t, 65], F16)
        sb_qT = singles.tile([128, 2, S], F16)
        sb_kT = singles.tile([128, 2, nkp], F16)
        sb_onT = singles.tile([128, 2, S], F16)

        # ---- DMA: minimize the critical prefix (kproj: wk + xkT chunk0;
        # qproj: wq + xT(0,qc)) by spreading over the 3 queues.
        nc.scalar.dma_start(out=sb_xkT[:, 0:4], in_=d_xkT[:, 0:4])
        nc.scalar.dma_start(out=sb_xT[:, 0, 1], in_=d_xT[:, 0, 1])
        for c in range(1, nkc):
            lo, hi = 4 * c, min(4 * c + 4, nk_t)
            nc.scalar.dma_start(out=sb_xkT[:, lo:hi], in_=d_xkT[:, lo:hi])
        nc.scalar.dma_start(out=sb_wo, in_=d_wo[:, :, :])
        nc.sync.dma_start(out=sb_xT[:, 0, 0], in_=d_xT[:, 0, 0])
        nc.sync.dma_start(out=sb_xT[:, 1, 0], in_=d_xT[:, 1, 0])
        nc.sync.dma_start(out=sb_xT[:, 1, 1], in_=d_xT[:, 1, 1])
        nc.gpsimd.dma_start(out=sb_wq, in_=d_wq[:, :, :])
        nc.gpsimd.dma_start(out=sb_wk, in_=d_wk[:, :, :])
        nc.gpsimd.dma_start(out=sb_wv, in_=d_wv[:, :, :])
        nc.gpsimd.dma_start(out=sb_bq, in_=d_bq[:, :])
        nc.gpsimd.dma_start(out=sb_bk, in_=d_bk[:, :])
        for h in range(NH):
            nc.gpsimd.dma_start(out=sb_v[:, h, :, 64:65], in_=d_vcol[:, :])

        stack2 = ctx.enter_context(ExitStack())
        stp = stack2.enter_context(tc.tile_pool(name="stp", bufs=2, space="PSUM"))
        oap = stack2.enter_context(tc.tile_pool(name="oap", bufs=1, space="PSUM"))
        pjp = stack2.enter_context(tc.tile_pool(name="pjp", bufs=2, space="PSUM"))

        # ---- projection / drain unit helpers ----
        def qproj_half(half, m, qc, ps, lo):
            # 3 contraction tiles; lo selects kt 0-2 (start) or 3-5 (stop)
            rows = 128 if m == 0 else 64
            msl = ds(m * 128, rows)
            for kt in range(3 * lo, 3 * lo + 3):
                nc.tensor.matmul(
                    ps[:rows, :],
                    sb_wq[:, kt, msl],
                    sb_xT[:, half, qc, kt, :],
                    start=(kt == 0),
                    stop=(kt == KT_D - 1),
                )
            if lo == 1:
                nc.vector.tensor_scalar_add(
                    out=sb_qT[:rows, m, ds(half * 1024 + qc * 512, 512)],
                    in0=ps[:rows, :],
                    scalar1=sb_bq[:rows, m : m + 1],
                )

        def kproj_half(m, c, ps, lo):
            rows = 128 if m == 0 else 64
            msl = ds(m * 128, rows)
            nkt_c = min(4 * c + 4, nk_t) - 4 * c
            for kt in range(3 * lo, 3 * lo + 3):
                nc.tensor.matmul(
                    ps[:rows, 0 : nkt_c * 128],
                    sb_wk[:, kt, msl],
                    sb_xkT[:, ds(4 * c, nkt_c), kt, :],
                    start=(kt == 0),
                    stop=(kt == KT_D - 1),
                )
            if lo == 1:
                nc.vector.tensor_scalar_add(
                    out=sb_kT[:rows, m, ds(c * 512, nkt_c * 128)],
                    in0=ps[:rows, 0 : nkt_c * 128],
                    scalar1=sb_bk[:rows, m : m + 1],
                )

        def vproj(kt2):
            ps = pjp.tile([128, 512], F32, tag="ps")
            for kt in range(KT_D):
                nc.tensor.matmul(
                    ps[:, 0:192],
                    sb_xkT[:, kt2, kt, :],
                    sb_wv[:, kt, :],
                    start=(kt == 0),
                    stop=(kt == KT_D - 1),
                )
            nc.vector.tensor_copy(
                out=sb_v[:, :, kt2, 0:64],
                in_=ps[:, 0:192].rearrange("p (h d) -> p h d", h=NH),
            )

        def oproj_chunk(qt, og, j, eoff, ech, pool, cast):
            ps = pool.tile([128, 512], F32, tag="ps")
            nc.tensor.matmul(
                ps[:, :ech],
                sb_onT[:, 0, ts(qt, 128)],
                sb_wo[:, 0, ds(eoff, ech)],
                start=True,
                stop=False,
            )
            nc.tensor.matmul(
                ps[:, :ech],
                sb_onT[0:64, 1, ts(qt, 128)],
                sb_wo[0:64, 1, ds(eoff, ech)],
                start=False,
                stop=True,
            )
            if cast == "scalar":
                nc.scalar.copy(out=og[:, j, ds(eoff, ech)], in_=ps[:, :ech])
            else:
                nc.vector.tensor_copy(out=og[:, j, ds(eoff, ech)], in_=ps[:, :ech])

        # ---- attention step helpers ----
        def head_rows(head):
            qrow = 64 if head == 1 else 0
            slot = 1 if head == 2 else 0
            return qrow, slot

        def qk(head, half, kt2, sT):
            qrow, slot = head_rows(head)
            for qc in range(2):
                nc.tensor.matmul(
                    sT[:, ts(qc, 512)],
                    sb_kT[ds(qrow, 64), slot, ts(kt2, 128)],
                    sb_qT[ds(qrow, 64), slot, ds(half * 1024 + qc * 512, 512)],
                    start=True,
                    stop=True,
                )

        def pv(head, kt2, et, oacc):
            for qc in range(2):
                nc.tensor.matmul(
                    oacc[:, ts(qc, 512)],
                    sb_v[:, head, kt2, :],
                    et[:, ts(qc, 512)],
                    start=(kt2 == 0),
                    stop=(kt2 == nk_t - 1),
                )

        def norm(head, half, oacc):
            qrow, slot = head_rows(head)
            for ch in range(2):
                csl = ds(ch * 512, 512)
                drow = rowp.tile([1, 512], F32, tag="drow")
                nc.vector.tensor_copy(out=drow, in_=oacc[64:65, csl])
                rrow = rowp.tile([1, 512], F32, tag="rrow")
                nc.vector.reciprocal_approx_fast(out=rrow, in_=drow)
                rb = bcp.tile([64, 512], F32, tag="rb")
                nc.gpsimd.partition_broadcast(rb, rrow)
                nc.vector.tensor_mul(
                    out=sb_onT[ds(qrow, 64), slot, ds(half * 1024 + ch * 512, 512)],
                    in0=oacc[0:64, csl],
                    in1=rb,
                )

        # ---- deferred unit schedule: at most one half-unit per step ----
        og_h0 = {}

        def oproj_h0(qt):
            if qt % 2 == 0:
                og_h0[qt // 2] = ogp.tile([128, 2, D], F16, tag="og", name="og")
            og = og_h0[qt // 2]
            j = qt % 2
            oproj_chunk(qt, og, j, 0, 512, pjp, "vector")
            oproj_chunk(qt, og, j, 512, 256, pjp, "vector")
            if qt % 2 == 1:
                nc.sync.dma_start(out=d_out[:, ds(qt - 1, 2), :], in_=og)

        units = {}

        def add_unit(step, th):
            units.setdefault(step, []).append(th)

        kp_ps = {}

        def kproj_u(m, c, lo):
            key = (m, c)
            if lo == 0:
                kp_ps[key] = pjp.tile([128, 512], F32, tag="ps", name="ps")
            kproj_half(m, c, kp_ps[key], lo)

        qp_ps = {}

        def qproj_u(half, m, qc, lo):
            key = (half, m, qc)
            if lo == 0:
                qp_ps[key] = pjp.tile([128, 512], F32, tag="ps", name="ps")
            qproj_half(half, m, qc, qp_ps[key], lo)

        # vproj(kt2) feeds PV(A, h0, kt2) one step later; bunch 0-3 into
        # steps 0-1 to free steps 2-3 for the kproj slot0 chunk-1 halves
        # (QK(A, kt2=4) needs kT cols 512+ at step 4).
        add_unit(0, lambda: vproj(0))
        add_unit(0, lambda: vproj(1))
        add_unit(1, lambda: vproj(2))
        add_unit(1, lambda: vproj(3))
        for kt2 in range(4, nk_t):
            add_unit(kt2, lambda k=kt2: vproj(k))
        s = 2
        for c in range(1, nkc):
            add_unit(s, lambda cc=c: kproj_u(0, cc, 0))
            add_unit(s + 1, lambda cc=c: kproj_u(0, cc, 1))
            s += 2
        # kproj m=1 (head C) as half-units; needed by C-h0 at 2*nk_t
        base = nk_t
        for c in range(nkc):
            add_unit(base + 2 * c, lambda cc=c: kproj_u(1, cc, 0))
            add_unit(base + 2 * c + 1, lambda cc=c: kproj_u(1, cc, 1))
        qb = base + 2 * nkc
        add_unit(qb + 0, lambda: qproj_u(0, 1, 0, 0))
        add_unit(qb + 1, lambda: qproj_u(0, 1, 0, 1))
        add_unit(qb + 2, lambda: qproj_u(0, 1, 1, 0))
        add_unit(qb + 3, lambda: qproj_u(0, 1, 1, 1))
        # during C-h0 (steps 2*nk_t..): q projections for half 1
        b2 = 2 * nk_t
        add_unit(b2 + 0, lambda: qproj_u(1, 0, 0, 0))
        add_unit(b2 + 1, lambda: qproj_u(1, 0, 0, 1))
        add_unit(b2 + 2, lambda: qproj_u(1, 0, 1, 0))
        add_unit(b2 + 3, lambda: qproj_u(1, 0, 1, 1))
        add_unit(b2 + 4, lambda: qproj_u(1, 1, 0, 0))
        add_unit(b2 + 5, lambda: qproj_u(1, 1, 0, 1))
        add_unit(b2 + 6, lambda: qproj_u(1, 1, 1, 0))
        add_unit(b2 + 7, lambda: qproj_u(1, 1, 1, 1))
        # oproj for half 0 (q tiles 0-7), one tile every other step during
        # half-1 attention
        b3 = 3 * nk_t
        for i in range(8):
            add_unit(b3 + 1 + 2 * i, lambda q=i: oproj_h0(q))

        # ---- lead-in: minimum to start (A, h0) ----
        kproj_u(0, 0, 0)
        kproj_u(0, 0, 1)
        qproj_u(0, 0, 0, 0)
        qproj_u(0, 0, 0, 1)
        qproj_u(0, 0, 1, 0)
        qproj_u(0, 0, 1, 1)

        # ---- 48-step attention pipeline ----
        order = [(0, 0), (0, 1), (0, 2), (1, 0), (1, 1), (1, 2)]
        prev = None
        step = 0
        for half, head in order:
            oacc = oap.tile([65, 1024], F32, tag="oacc")
            for kt2 in range(nk_t):
                sT = stp.tile([128, 1024], F32, tag="sT")
                qk(head, half, kt2, sT)
                et = etp.tile([128, 1024], F16, tag="et")
                nc.scalar.activation(et, sT, AFT.Exp, bias=sb_soff[:, 0:1])
                if prev is not None:
                    ph, phalf, pkt2, pet, poacc = prev
                    pv(ph, pkt2, pet, poacc)
                    if pkt2 == nk_t - 1:
                        norm(ph, phalf, poacc)
                for th in units.pop(step, []):
                    th()
                prev = (head, half, kt2, et, oacc)
                step += 1
        ph, phalf, pkt2, pet, poacc = prev
        pv(ph, pkt2, pet, poacc)
        norm(ph, phalf, poacc)
        for s in sorted(units):
            for th in units.pop(s):
                th()

        # close attention PSUM pools; tail gets its own 4-buffer pool
        stack2.close()
        tailp = ctx.enter_context(tc.tile_pool(name="tailp", bufs=4, space="PSUM"))

        # ---- tail: oproj for half 1 (q tiles 8-15) ----
        for i in range(4):
            og = ogp.tile([128, 2, D], F16, tag="og")
            for j in range(2):
                qt = 8 + 2 * i + j
                oproj_chunk(qt, og, j, 0, 512, tailp, "scalar")
                oproj_chunk(qt, og, j, 512, 256, tailp, "vector")
            eng = nc.sync if i % 2 == 0 else nc.scalar
            eng.dma_start(out=d_out[:, ds(8 + 2 * i, 2), :], in_=og)

    nc.compile()
    return nc


def kernel(
    hidden_states,
    complexity_scores,
    attention_mask,
    Wq,
    bq,
    Wk,
    bk,
    Wv,
    bv,
    Wo,
    bo,
    emb_table,
    comp_scaling,
):
    global LAST_EXEC_TIME_NS, LAST_RESULTS
    hs = np.asarray(hidden_states, np.float32)
    cs = np.asarray(complexity_scores).astype(np.int64)
    am = np.asarray(attention_mask)
    Wq = np.asarray(Wq, np.float32)
    bq = np.asarray(bq, np.float32)
    Wk = np.asarray(Wk, np.float32)
    bk = np.asarray(bk, np.float32)
    Wv = np.asarray(Wv, np.float32)
    bv = np.asarray(bv, np.float32)
    Wo = np.asarray(Wo, np.float32)
    bo = np.asarray(bo, np.float32)
    emb_table = np.asarray(emb_table, np.float32)
    comp_scaling = np.asarray(comp_scaling, np.float32)

    # per-head score scale (identical across batch: mean over batch of embs)
    embs = emb_table[cs]  # (B, H)
    scal = comp_scaling * embs.mean(axis=0)  # (H,)
    c = (scal / math.sqrt(HD)).astype(np.float32)

    # gather unmasked keys per batch; pad to a common multiple of 128
    idx = [np.nonzero(am[b] != 0)[0] for b in range(B)]
    n_max = max(1, max(len(i) for i in idx))
    nk_t = max(2, (n_max + 127) // 128)
    n_k = nk_t * 128
    nkc = (n_k + 511) // 512
    nkp = nkc * 512

    xT = []
    xkT = []
    vcol = []
    for b in range(B):
        t = hs[b].T.astype(np.float16)  # (768, 2048)
        # (128, half, qc, kt, 512)
        xT.append(
            np.ascontiguousarray(
                t.reshape(KT_D, 128, 2, 2, 512).transpose(1, 2, 3, 0, 4)
            )
        )
        tk = np.zeros((D, nk_t * 128), np.float16)
        tk[:, : len(idx[b])] = hs[b][idx[b]].T
        # (128, nk_t, KT_D, 128): key-tile-major, contiguous per partition
        xkT.append(
            np.ascontiguousarray(
                tk.reshape(KT_D, 128, nk_t, 128).transpose(1, 2, 0, 3)
            )
        )
        v = np.zeros((nk_t * 128,), np.float16)
        v[: len(idx[b])] = 1.0
        vcol.append(np.ascontiguousarray(v.reshape(nk_t, 128).T))

    WqT = Wq.T  # (d_in, e_out)
    WkT = Wk.T
    WvT = Wv.T
    WoT = np.ascontiguousarray(Wo.T)  # rows = attended feature d

    def pack_w(w192):  # (768, 192) -> (128, KT_D, 192)
        return np.ascontiguousarray(
            w192.astype(np.float16).reshape(KT_D, 128, 192).transpose(1, 0, 2)
        )

    def pack_bias(vec):  # (192,) -> (128, 2)
        out = np.zeros((128, 2), np.float32)
        out[:, 0] = vec[:128]
        out[:64, 1] = vec[128:]
        return out

    in_maps = []
    for core in range(8):
        b = core // 4
        heads = [3 * (core % 4) + j for j in range(NH)]
        cols = np.concatenate([np.arange(h * HD, (h + 1) * HD) for h in heads])
        cscale = np.repeat(c[heads], HD)  # (192,)
        wq_c = pack_w(WqT[:, cols] * cscale[None, :])
        bq_c = bq[cols] * cscale
        wk_c = pack_w(WkT[:, cols])
        bk_c = bk[cols]
        wv_c = pack_w(WvT[:, cols])
        wo_c = np.zeros((128, 2, D), np.float16)
        wo_c[:, 0, :] = WoT[cols[:128], :]
        wo_c[:64, 1, :] = WoT[cols[128:], :]
        in_maps.append(
            {
                "xT": xT[b],
                "xkT": xkT[b],
                "wq": wq_c,
                "wk": wk_c,
                "wv": wv_c,
                "wo": np.ascontiguousarray(wo_c),
                "bq": pack_bias(bq_c),
                "bk": pack_bias(bk_c),
                "vcol": vcol[b],
            }
        )

    nc = build_nc(nk_t)
    trace = os.environ.get("KERNEL_TRACE", "0") == "1"
    res = run_bass_kernel_spmd(nc, in_maps, core_ids=list(range(8)), trace=trace)
    LAST_EXEC_TIME_NS = res.exec_time_ns
    LAST_RESULTS = res

    bo_eff = (bo + Wo @ bv).astype(np.float64)
    out = np.empty((B, S, D), np.float32)
    for b in range(B):
        acc = np.zeros((S, D), np.float64)
        for g in range(4):
            p = res.results[4 * b + g]["out"]  # (128, 16, D) fp16
            acc += p.astype(np.float64).transpose(1, 0, 2).reshape(S, D)
        out[b] = (acc + bo_eff[None, :]).astype(np.float32)
    return out


# revision 29
# speedup vs baseline: 1.0971x; 1.0119x over previous
"""ComplexityAwareAttention Trainium2 Bass kernel (v3 schedule).

Sharding: 8 cores = 2 batches x 4 head-groups (3 heads each). Each core
computes q/k/v projections for its 3 heads, masked-key-gathered attention
(keys with attention_mask==0 are removed on host), and a partial output
projection (2048, 768). Host sums the 4 partials per batch and adds the
fused output bias (bo + Wo @ bv).

v3 schedule: paced by ScalarE's exp stream (48 tiles of [128,1024]).
Lead-in is DMA-critical-path minimized (4 parallel queues, kproj starts
on a 128-key slice to warm the PE early). All other projection work is
spread as <=3-matmul half-units, one per attention step, so the PE feeds
the exp stream without burst gaps. Softmax normalize: copy denom row ->
reciprocal_approx_fast -> Pool partition_broadcast -> full-width DVE
multiply (512-col chunks; wider PSUM APs cross banks and misbehave in
the custom DVE op). Tail: attention PSUM pools are closed and a
4-buffer tail pool ping-pongs the half-1 output projection with casts
split across ScalarE and DVE.

PSUM budget (8 banks): sT double-buffer (4) + oacc (2) + projection
ping-pong (2); tail reuses 4 banks after the attention pools close.
"""

import math
import os
from contextlib import ExitStack

import numpy as np

import concourse.bass as bass
from concourse import bacc
import concourse.mybir as mybir
import concourse.tile as tile
from concourse.bass import ds, ts
from concourse.bass_utils import run_bass_kernel_spmd

F32 = mybir.dt.float32
F16 = mybir.dt.float16
AFT = mybir.ActivationFunctionType

B = 2
S = 2048
D = 768
H = 12
HD = 64
NH = 3  # heads per core
KT_D = D // 128  # 6 contraction tiles over d_model
SCORE_OFF = 12.5  # subtracted inside exp so et fits fp16 (scores reach ~22)

LAST_EXEC_TIME_NS = None
LAST_RESULTS = None


def build_nc(nk_t):
    n_k = nk_t * 128
    nkc = (n_k + 511) // 512  # xkT 512-col chunks
    nkp = nkc * 512  # padded key columns
    nc = bacc.Bacc(None, target_bir_lowering=False)

    # xT packed as (128, half, qc, kt, 512) so each (half, qc) projection
    # unit's DMA slice is contiguous per partition.
    d_xT = nc.dram_tensor("xT", (128, 2, 2, KT_D, 512), F16, kind="ExternalInput")
    # key-tile-major so every DMA prefix is contiguous per partition
    d_xkT = nc.dram_tensor("xkT", (128, nk_t, KT_D, 128), F16, kind="ExternalInput")
    d_wq = nc.dram_tensor("wq", (128, KT_D, 192), F16, kind="ExternalInput")
    d_wk = nc.dram_tensor("wk", (128, KT_D, 192), F16, kind="ExternalInput")
    d_wv = nc.dram_tensor("wv", (128, KT_D, 192), F16, kind="ExternalInput")
    d_wo = nc.dram_tensor("wo", (128, 2, D), F16, kind="ExternalInput")
    d_bq = nc.dram_tensor("bq", (128, 2), F32, kind="ExternalInput")
    d_bk = nc.dram_tensor("bk", (128, 2), F32, kind="ExternalInput")
    d_vcol = nc.dram_tensor("vcol", (128, nk_t), F16, kind="ExternalInput")
    d_out = nc.dram_tensor("out", (128, 16, D), F16, kind="ExternalOutput")

    with ExitStack() as ctx:
        tc = ctx.enter_context(tile.TileContext(nc))
        singles = ctx.enter_context(tc.tile_pool(name="singles", bufs=1))
        etp = ctx.enter_context(tc.tile_pool(name="etp", bufs=6))
        rowp = ctx.enter_context(tc.tile_pool(name="rowp", bufs=2))
        bcp = ctx.enter_context(tc.tile_pool(name="bcp", bufs=2))
        ogp = ctx.enter_context(tc.tile_pool(name="ogp", bufs=4))

        sb_xT = singles.tile([128, 2, 2, KT_D, 512], F16)
        sb_xkT = singles.tile([128, nk_t, KT_D, 128], F16)
        sb_wq = singles.tile([128, KT_D, 192], F16)
        sb_wk = singles.tile([128, KT_D, 192], F16)
        sb_wv = singles.tile([128, KT_D, 192], F16)
        sb_wo = singles.tile([128, 2, D], F16)
        sb_bq = singles.tile([128, 2], F32)
        sb_bk = singles.tile([128, 2], F32)
        sb_v = singles.tile([128, NH, nk_t, 65], F16)
        sb_qT = singles.tile([128, 2, S], F16)
        sb_kT = singles.tile([128, 2, nkp], F16)
        sb_onT = singles.tile([128, 2, S], F16)

        # ---- DMA: emitted first so queue kicks precede the ACT table
        # load; critical-first order (exp0 needs wk, wq, xkT chunk0,
        # xT(0,*)); xT(0,1) is split across the two HW queues.
        nc.scalar.dma_start(out=sb_xkT[:, 0:4], in_=d_xkT[:, 0:4])
        nc.scalar.dma_start(out=sb_xT[:, 0, 1, 3:6], in_=d_xT[:, 0, 1, 3:6])
        for c in range(1, nkc):
            lo, hi = 4 * c, min(4 * c + 4, nk_t)
            nc.scalar.dma_start(out=sb_xkT[:, lo:hi], in_=d_xkT[:, lo:hi])
        nc.scalar.dma_start(out=sb_wo, in_=d_wo[:, :, :])
        nc.sync.dma_start(out=sb_xT[:, 0, 0], in_=d_xT[:, 0, 0])
        nc.sync.dma_start(out=sb_xT[:, 0, 1, 0:3], in_=d_xT[:, 0, 1, 0:3])
        nc.sync.dma_start(out=sb_xT[:, 1, 0], in_=d_xT[:, 1, 0])
        nc.sync.dma_start(out=sb_xT[:, 1, 1], in_=d_xT[:, 1, 1])
        nc.gpsimd.dma_start(out=sb_wq, in_=d_wq[:, :, :])
        nc.gpsimd.dma_start(out=sb_wk, in_=d_wk[:, :, :])
        nc.gpsimd.dma_start(out=sb_wv, in_=d_wv[:, :, :])
        nc.gpsimd.dma_start(out=sb_bq, in_=d_bq[:, :])
        nc.gpsimd.dma_start(out=sb_bk, in_=d_bk[:, :])
        for h in range(NH):
            nc.gpsimd.dma_start(out=sb_v[:, h, :, 64:65], in_=d_vcol[:, :])

        # Pull the Exp activation table load off the critical path.
        dummy = singles.tile([1, 2], F32)
        nc.vector.memset(dummy, 0.0)
        nc.scalar.activation(dummy, dummy, AFT.Exp)
        # per-partition exp bias (score offset; cancels in softmax)
        sb_soff = singles.tile([128, 1], F32)
        nc.vector.memset(sb_soff, -SCORE_OFF)

        stack2 = ctx.enter_context(ExitStack())
        stp = stack2.enter_context(tc.tile_pool(name="stp", bufs=2, space="PSUM"))
        oap = stack2.enter_context(tc.tile_pool(name="oap", bufs=1, space="PSUM"))
        pjp = stack2.enter_context(tc.tile_pool(name="pjp", bufs=2, space="PSUM"))

        # ---- projection / drain unit helpers ----
        def qproj_half(half, m, qc, ps, lo):
            # 3 contraction tiles; lo selects kt 0-2 (start) or 3-5 (stop)
            rows = 128 if m == 0 else 64
            msl = ds(m * 128, rows)
            for kt in range(3 * lo, 3 * lo + 3):
                nc.tensor.matmul(
                    ps[:rows, :],
                    sb_wq[:, kt, msl],
                    sb_xT[:, half, qc, kt, :],
                    start=(kt == 0),
                    stop=(kt == KT_D - 1),
                )
            if lo == 1:
                nc.vector.tensor_scalar_add(
                    out=sb_qT[:rows, m, ds(half * 1024 + qc * 512, 512)],
                    in0=ps[:rows, :],
                    scalar1=sb_bq[:rows, m : m + 1],
                )

        def kproj_half(m, c, ps, lo):
            rows = 128 if m == 0 else 64
            msl = ds(m * 128, rows)
            nkt_c = min(4 * c + 4, nk_t) - 4 * c
            for kt in range(3 * lo, 3 * lo + 3):
                nc.tensor.matmul(
                    ps[:rows, 0 : nkt_c * 128],
                    sb_wk[:, kt, msl],
                    sb_xkT[:, ds(4 * c, nkt_c), kt, :],
                    start=(kt == 0),
                    stop=(kt == KT_D - 1),
                )
            if lo == 1:
                nc.vector.tensor_scalar_add(
                    out=sb_kT[:rows, m, ds(c * 512, nkt_c * 128)],
                    in0=ps[:rows, 0 : nkt_c * 128],
                    scalar1=sb_bk[:rows, m : m + 1],
                )

        def vproj(kt2):
            ps = pjp.tile([128, 512], F32, tag="ps")
            for kt in range(KT_D):
                nc.tensor.matmul(
                    ps[:, 0:192],
                    sb_xkT[:, kt2, kt, :],
                    sb_wv[:, kt, :],
                    start=(kt == 0),
                    stop=(kt == KT_D - 1),
                )
            nc.vector.tensor_copy(
                out=sb_v[:, :, kt2, 0:64],
                in_=ps[:, 0:192].rearrange("p (h d) -> p h d", h=NH),
            )

        def oproj_chunk(qt, og, j, eoff, ech, pool, cast):
            ps = pool.tile([128, 512], F32, tag="ps")
            nc.tensor.matmul(
                ps[:, :ech],
                sb_onT[:, 0, ts(qt, 128)],
                sb_wo[:, 0, ds(eoff, ech)],
                start=True,
                stop=False,
            )
            nc.tensor.matmul(
                ps[:, :ech],
                sb_onT[0:64, 1, ts(qt, 128)],
                sb_wo[0:64, 1, ds(eoff, ech)],
                start=False,
                stop=True,
            )
            if cast == "scalar":
                nc.scalar.copy(out=og[:, j, ds(eoff, ech)], in_=ps[:, :ech])
            else:
                nc.vector.tensor_copy(out=og[:, j, ds(eoff, ech)], in_=ps[:, :ech])

        # ---- attention step helpers ----
        def head_rows(head):
            qrow = 64 if head == 1 else 0
            slot = 1 if head == 2 else 0
            return qrow, slot

        def qk(head, half, kt2, sT):
            qrow, slot = head_rows(head)
            for qc in range(2):
                nc.tensor.matmul(
                    sT[:, ts(qc, 512)],
                    sb_kT[ds(qrow, 64), slot, ts(kt2, 128)],
                    sb_qT[ds(qrow, 64), slot, ds(half * 1024 + qc * 512, 512)],
                    start=True,
                    stop=True,
                )

        def pv(head, kt2, et, oacc):
            for qc in range(2):
                nc.tensor.matmul(
                    oacc[:, ts(qc, 512)],
                    sb_v[:, head, kt2, :],
                    et[:, ts(qc, 512)],
                    start=(kt2 == 0),
                    stop=(kt2 == nk_t - 1),
                )

        def norm(head, half, oacc):
            # evacuate oacc to SBUF first (frees the single PSUM oacc for
            # the next head's PV with ~1.5us latency), then normalize
            # entirely SBUF-side off the critical path.
            qrow, slot = head_rows(head)
            for ch in range(2):
                csl = ds(ch * 512, 512)
                drow = rowp.tile([1, 512], F32, tag="drow")
                nc.vector.tensor_copy(out=drow, in_=oacc[64:65, csl])
                rrow = rowp.tile([1, 512], F32, tag="rrow")
                nc.vector.reciprocal_approx_fast(out=rrow, in_=drow)
                rb = bcp.tile([64, 512], F32, tag="rb")
                nc.gpsimd.partition_broadcast(rb, rrow)
                nc.vector.tensor_mul(
                    out=sb_onT[ds(qrow, 64), slot, ds(half * 1024 + ch * 512, 512)],
                    in0=oacc[0:64, csl],
                    in1=rb,
                )

        # ---- deferred unit schedule: at most one half-unit per step ----
        og_h0 = {}

        def oproj_h0(qt):
            if qt % 2 == 0:
                og_h0[qt // 2] = ogp.tile([128, 2, D], F16, tag="og", name="og")
            og = og_h0[qt // 2]
            j = qt % 2
            oproj_chunk(qt, og, j, 0, 512, pjp, "vector")
            oproj_chunk(qt, og, j, 512, 256, pjp, "vector")
            if qt % 2 == 1:
                nc.sync.dma_start(out=d_out[:, ds(qt - 1, 2), :], in_=og)

        units = {}

        def add_unit(step, th):
            units.setdefault(step, []).append(th)

        kp_ps = {}

        def kproj_u(m, c, lo):
            key = (m, c)
            if lo == 0:
                kp_ps[key] = pjp.tile([128, 512], F32, tag="ps", name="ps")
            kproj_half(m, c, kp_ps[key], lo)

        qp_ps = {}

        def qproj_u(half, m, qc, lo):
            key = (half, m, qc)
            if lo == 0:
                qp_ps[key] = pjp.tile([128, 512], F32, tag="ps", name="ps")
            qproj_half(half, m, qc, qp_ps[key], lo)

        # vproj(kt2) feeds PV(A, h0, kt2) one step later; bunch 0-3 into
        # steps 0-1 to free steps 2-3 for the kproj slot0 chunk-1 halves
        # (QK(A, kt2=4) needs kT cols 512+ at step 4).
        add_unit(0, lambda: vproj(0))
        add_unit(0, lambda: vproj(1))
        add_unit(1, lambda: vproj(2))
        add_unit(1, lambda: vproj(3))
        for kt2 in range(4, nk_t):
            add_unit(kt2, lambda k=kt2: vproj(k))
        s = 2
        for c in range(1, nkc):
            add_unit(s, lambda cc=c: kproj_u(0, cc, 0))
            add_unit(s + 1, lambda cc=c: kproj_u(0, cc, 1))
            s += 2
        # kproj m=1 (head C) as half-units; needed by C-h0 at 2*nk_t
        base = nk_t
        for c in range(nkc):
            add_unit(base + 2 * c, lambda cc=c: kproj_u(1, cc, 0))
            add_unit(base + 2 * c + 1, lambda cc=c: kproj_u(1, cc, 1))
        qb = base + 2 * nkc
        add_unit(qb + 0, lambda: qproj_u(0, 1, 0, 0))
        add_unit(qb + 1, lambda: qproj_u(0, 1, 0, 1))
        add_unit(qb + 2, lambda: qproj_u(0, 1, 1, 0))
        add_unit(qb + 3, lambda: qproj_u(0, 1, 1, 1))
        # during C-h0 (steps 2*nk_t..): q projections for half 1
        b2 = 2 * nk_t
        add_unit(b2 + 0, lambda: qproj_u(1, 0, 0, 0))
        add_unit(b2 + 1, lambda: qproj_u(1, 0, 0, 1))
        add_unit(b2 + 2, lambda: qproj_u(1, 0, 1, 0))
        add_unit(b2 + 3, lambda: qproj_u(1, 0, 1, 1))
        add_unit(b2 + 4, lambda: qproj_u(1, 1, 0, 0))
        add_unit(b2 + 5, lambda: qproj_u(1, 1, 0, 1))
        add_unit(b2 + 6, lambda: qproj_u(1, 1, 1, 0))
        add_unit(b2 + 7, lambda: qproj_u(1, 1, 1, 1))
        # oproj for half 0 (q tiles 0-7), one tile every other step during
        # half-1 attention (offset +3 so the C-h0 normalize has landed)
        b3 = 3 * nk_t
        for i in range(8):
            add_unit(min(b3 + 3 + 2 * i, 6 * nk_t - 1), lambda q=i: oproj_h0(q))

        # ---- lead-in: minimum to start (A, h0) ----
        kproj_u(0, 0, 0)
        kproj_u(0, 0, 1)
        qproj_u(0, 0, 0, 0)
        qproj_u(0, 0, 0, 1)
        qproj_u(0, 0, 1, 0)
        qproj_u(0, 0, 1, 1)

        # ---- 48-step attention pipeline ----
        order = [(0, 0), (0, 1), (0, 2), (1, 0), (1, 1), (1, 2)]
        prev = None
        step = 0
        for half, head in order:
            oacc = oap.tile([65, 1024], F32, tag="oacc")
            for kt2 in range(nk_t):
                sT = stp.tile([128, 1024], F32, tag="sT")
                qk(head, half, kt2, sT)
                et = etp.tile([128, 1024], F16, tag="et")
                nc.scalar.activation(et, sT, AFT.Exp, bias=sb_soff[:, 0:1])
                if prev is not None:
                    ph, phalf, pkt2, pet, poacc = prev
                    pv(ph, pkt2, pet, poacc)
                    if pkt2 == nk_t - 1:
                        norm(ph, phalf, poacc)
                for th in units.pop(step, []):
                    th()
                prev = (head, half, kt2, et, oacc)
                step += 1
        ph, phalf, pkt2, pet, poacc = prev
        pv(ph, pkt2, pet, poacc)
        norm(ph, phalf, poacc)
        for s in sorted(units):
            for th in units.pop(s):
                th()

        # keep the PE HAM-warm through the final normalize window: a few
        # throwaway matmuls into a scratch bank (result never read)
        if False:
            warm = pjp.tile([128, 512], F32, tag="ps", name="warm")
            for _ in range(5):
                nc.tensor.matmul(
                    warm, sb_wq[:, 0, 0:128], sb_qT[:, 0, 0:512], start=True, stop=True
                )

        # close attention PSUM pools; tail gets its own 4-buffer pool
        stack2.close()
        tailp = ctx.enter_context(tc.tile_pool(name="tailp", bufs=4, space="PSUM"))

        # ---- tail: oproj for half 1 (q tiles 8-15) ----
        for i in range(4):
            og = ogp.tile([128, 2, D], F16, tag="og")
            for j in range(2):
                qt = 8 + 2 * i + j
                oproj_chunk(qt, og, j, 0, 512, tailp, "scalar")
                oproj_chunk(qt, og, j, 512, 256, tailp, "vector")
            eng = nc.sync if i % 2 == 0 else nc.scalar
            eng.dma_start(out=d_out[:, ds(8 + 2 * i, 2), :], in_=og)

    nc.compile()
    return nc


def kernel(
    hidden_states,
    complexity_scores,
    attention_mask,
    Wq,
    bq,
    Wk,
    bk,
    Wv,
    bv,
    Wo,
    bo,
    emb_table,
    comp_scaling,
):
    global LAST_EXEC_TIME_NS, LAST_RESULTS
    hs = np.asarray(hidden_states, np.float32)
    cs = np.asarray(complexity_scores).astype(np.int64)
    am = np.asarray(attention_mask)
    Wq = np.asarray(Wq, np.float32)
    bq = np.asarray(bq, np.float32)
    Wk = np.asarray(Wk, np.float32)
    bk = np.asarray(bk, np.float32)
    Wv = np.asarray(Wv, np.float32)
    bv = np.asarray(bv, np.float32)
    Wo = np.asarray(Wo, np.float32)
    bo = np.asarray(bo, np.float32)
    emb_table = np.asarray(emb_table, np.float32)
    comp_scaling = np.asarray(comp_scaling, np.float32)

    # per-head score scale (identical across batch: mean over batch of embs)
    embs = emb_table[cs]  # (B, H)
    scal = comp_scaling * embs.mean(axis=0)  # (H,)
    c = (scal / math.sqrt(HD)).astype(np.float32)

    # gather unmasked keys per batch; pad to a common multiple of 128
    idx = [np.nonzero(am[b] != 0)[0] for b in range(B)]
    n_max = max(1, max(len(i) for i in idx))
    nk_t = max(2, (n_max + 127) // 128)
    n_k = nk_t * 128
    nkc = (n_k + 511) // 512
    nkp = nkc * 512

    xT = []
    xkT = []
    vcol = []
    for b in range(B):
        t = hs[b].T.astype(np.float16)  # (768, 2048)
        # (128, half, qc, kt, 512)
        xT.append(
            np.ascontiguousarray(
                t.reshape(KT_D, 128, 2, 2, 512).transpose(1, 2, 3, 0, 4)
            )
        )
        tk = np.zeros((D, nk_t * 128), np.float16)
        tk[:, : len(idx[b])] = hs[b][idx[b]].T
        # (128, nk_t, KT_D, 128): key-tile-major, contiguous per partition
        xkT.append(
            np.ascontiguousarray(
                tk.reshape(KT_D, 128, nk_t, 128).transpose(1, 2, 0, 3)
            )
        )
        v = np.zeros((nk_t * 128,), np.float16)
        v[: len(idx[b])] = 1.0
        vcol.append(np.ascontiguousarray(v.reshape(nk_t, 128).T))

    WqT = Wq.T  # (d_in, e_out)
    WkT = Wk.T
    WvT = Wv.T
    WoT = np.ascontiguousarray(Wo.T)  # rows = attended feature d

    def pack_w(w192):  # (768, 192) -> (128, KT_D, 192)
        return np.ascontiguousarray(
            w192.astype(np.float16).reshape(KT_D, 128, 192).transpose(1, 0, 2)
        )

    def pack_bias(vec):  # (192,) -> (128, 2)
        out = np.zeros((128, 2), np.float32)
        out[:, 0] = vec[:128]
        out[:64, 1] = vec[128:]
        return out

    in_maps = []
    for core in range(8):
        b = core // 4
        heads = [3 * (core % 4) + j for j in range(NH)]
        cols = np.concatenate([np.arange(h * HD, (h + 1) * HD) for h in heads])
        cscale = np.repeat(c[heads], HD)  # (192,)
        wq_c = pack_w(WqT[:, cols] * cscale[None, :])
        bq_c = bq[cols] * cscale
        wk_c = pack_w(WkT[:, cols])
        bk_c = bk[cols]
        wv_c = pack_w(WvT[:, cols])
        wo_c = np.zeros((128, 2, D), np.float16)
        wo_c[:, 0, :] = WoT[cols[:128], :]
        wo_c[:64, 1, :] = WoT[cols[128:], :]
        in_maps.append(
            {
                "xT": xT[b],
                "xkT": xkT[b],
                "wq": wq_c,
                "wk": wk_c,
                "wv": wv_c,
                "wo": np.ascontiguousarray(wo_c),
                "bq": pack_bias(bq_c),
                "bk": pack_bias(bk_c),
                "vcol": vcol[b],
            }
        )

    nc = build_nc(nk_t)
    trace = os.environ.get("KERNEL_TRACE", "0") == "1"
    res = run_bass_kernel_spmd(nc, in_maps, core_ids=list(range(8)), trace=trace)
    LAST_EXEC_TIME_NS = res.exec_time_ns
    LAST_RESULTS = res

    bo_eff = (bo + Wo @ bv).astype(np.float64)
    out = np.empty((B, S, D), np.float32)
    for b in range(B):
        acc = np.zeros((S, D), np.float64)
        for g in range(4):
            p = res.results[4 * b + g]["out"]  # (128, 16, D) fp16
            acc += p.astype(np.float64).transpose(1, 0, 2).reshape(S, D)
        out[b] = (acc + bo_eff[None, :]).astype(np.float32)
    return out


# revision 30
# speedup vs baseline: 1.1339x; 1.0336x over previous
"""ComplexityAwareAttention Trainium2 Bass kernel (v3 schedule).

Sharding: 8 cores = 2 batches x 4 head-groups (3 heads each). Each core
computes q/k/v projections for its 3 heads, masked-key-gathered attention
(keys with attention_mask==0 are removed on host), and a partial output
projection (2048, 768). Host sums the 4 partials per batch and adds the
fused output bias (bo + Wo @ bv).

v3 schedule: paced by ScalarE's exp stream (48 tiles of [128,1024]).
Lead-in is DMA-critical-path minimized (4 parallel queues, kproj starts
on a 128-key slice to warm the PE early). All other projection work is
spread as <=3-matmul half-units, one per attention step, so the PE feeds
the exp stream without burst gaps. Softmax normalize: copy denom row ->
reciprocal_approx_fast -> Pool partition_broadcast -> full-width DVE
multiply (512-col chunks; wider PSUM APs cross banks and misbehave in
the custom DVE op). Tail: attention PSUM pools are closed and a
4-buffer tail pool ping-pongs the half-1 output projection with casts
split across ScalarE and DVE.

PSUM budget (8 banks): sT double-buffer (4) + oacc (2) + projection
ping-pong (2); tail reuses 4 banks after the attention pools close.
"""

import math
import os
from contextlib import ExitStack

import numpy as np

import concourse.bass as bass
from concourse import bacc
import concourse.mybir as mybir
import concourse.tile as tile
from concourse.bass import ds, ts
from concourse.bass_utils import run_bass_kernel_spmd

F32 = mybir.dt.float32
F16 = mybir.dt.float16
AFT = mybir.ActivationFunctionType

B = 2
S = 2048
D = 768
H = 12
HD = 64
NH = 3  # heads per core
KT_D = D // 128  # 6 contraction tiles over d_model
SCORE_OFF = 12.5  # subtracted inside exp so et fits fp16 (scores reach ~22)

LAST_EXEC_TIME_NS = None
LAST_RESULTS = None


def build_nc(nk_t):
    n_k = nk_t * 128
    nkc = (n_k + 511) // 512  # xkT 512-col chunks
    nkp = nkc * 512  # padded key columns
    nc = bacc.Bacc(None, target_bir_lowering=False)

    # xT packed as (128, half, qc, kt, 512) so each (half, qc) projection
    # unit's DMA slice is contiguous per partition.
    d_xT = nc.dram_tensor("xT", (128, 2, 2, KT_D, 512), F16, kind="ExternalInput")
    # key-tile-major so every DMA prefix is contiguous per partition
    d_xkT = nc.dram_tensor("xkT", (128, nk_t, KT_D, 128), F16, kind="ExternalInput")
    d_wq = nc.dram_tensor("wq", (128, KT_D, 192), F16, kind="ExternalInput")
    d_wk = nc.dram_tensor("wk", (128, KT_D, 192), F16, kind="ExternalInput")
    d_wv = nc.dram_tensor("wv", (128, KT_D, 192), F16, kind="ExternalInput")
    d_wo = nc.dram_tensor("wo", (128, 2, D), F16, kind="ExternalInput")
    d_bq = nc.dram_tensor("bq", (128, 2), F32, kind="ExternalInput")
    d_bk = nc.dram_tensor("bk", (128, 2), F32, kind="ExternalInput")
    d_vcol = nc.dram_tensor("vcol", (128, nk_t), F16, kind="ExternalInput")
    d_out = nc.dram_tensor("out", (128, 16, D), F16, kind="ExternalOutput")

    with ExitStack() as ctx:
        tc = ctx.enter_context(tile.TileContext(nc))
        singles = ctx.enter_context(tc.tile_pool(name="singles", bufs=1))
        etp = ctx.enter_context(tc.tile_pool(name="etp", bufs=6))
        rowp = ctx.enter_context(tc.tile_pool(name="rowp", bufs=2))
        bcp = ctx.enter_context(tc.tile_pool(name="bcp", bufs=2))
        ogp = ctx.enter_context(tc.tile_pool(name="ogp", bufs=4))

        sb_xT = singles.tile([128, 2, 2, KT_D, 512], F16)
        sb_xkT = singles.tile([128, nk_t, KT_D, 128], F16)
        sb_wq = singles.tile([128, KT_D, 192], F16)
        sb_wk = singles.tile([128, KT_D, 192], F16)
        sb_wv = singles.tile([128, KT_D, 192], F16)
        sb_wo = singles.tile([128, 2, D], F16)
        sb_bq = singles.tile([128, 2], F32)
        sb_bk = singles.tile([128, 2], F32)
        sb_v = singles.tile([128, NH, nk_t, 65], F16)
        sb_qT = singles.tile([128, 2, S], F16)
        sb_kT = singles.tile([128, 2, nkp], F16)
        sb_onT = singles.tile([128, 2, S], F16)

        # ---- DMA: emitted first so queue kicks precede the ACT table
        # load; critical-first order (exp0 needs wk, wq, xkT chunk0,
        # xT(0,*)); xT(0,1) is split across the two HW queues.
        nc.scalar.dma_start(out=sb_xkT[:, 0:4], in_=d_xkT[:, 0:4])
        nc.scalar.dma_start(out=sb_xT[:, 0, 1, 3:6], in_=d_xT[:, 0, 1, 3:6])
        for c in range(1, nkc):
            lo, hi = 4 * c, min(4 * c + 4, nk_t)
            nc.scalar.dma_start(out=sb_xkT[:, lo:hi], in_=d_xkT[:, lo:hi])
        nc.scalar.dma_start(out=sb_wo, in_=d_wo[:, :, :])
        nc.sync.dma_start(out=sb_xT[:, 0, 0], in_=d_xT[:, 0, 0])
        nc.sync.dma_start(out=sb_xT[:, 0, 1, 0:3], in_=d_xT[:, 0, 1, 0:3])
        nc.sync.dma_start(out=sb_xT[:, 1, 0], in_=d_xT[:, 1, 0])
        nc.sync.dma_start(out=sb_xT[:, 1, 1], in_=d_xT[:, 1, 1])
        nc.gpsimd.dma_start(out=sb_wq, in_=d_wq[:, :, :])
        nc.gpsimd.dma_start(out=sb_wk, in_=d_wk[:, :, :])
        nc.gpsimd.dma_start(out=sb_wv, in_=d_wv[:, :, :])
        nc.gpsimd.dma_start(out=sb_bq, in_=d_bq[:, :])
        nc.gpsimd.dma_start(out=sb_bk, in_=d_bk[:, :])
        for h in range(NH):
            nc.gpsimd.dma_start(out=sb_v[:, h, :, 64:65], in_=d_vcol[:, :])

        # Pull the Exp activation table load off the critical path.
        dummy = singles.tile([1, 2], F32)
        nc.vector.memset(dummy, 0.0)
        nc.scalar.activation(dummy, dummy, AFT.Exp)
        # per-partition exp bias (score offset; cancels in softmax)
        sb_soff = singles.tile([128, 1], F32)
        nc.vector.memset(sb_soff, -SCORE_OFF)

        stack2 = ctx.enter_context(ExitStack())
        stp = stack2.enter_context(tc.tile_pool(name="stp", bufs=2, space="PSUM"))
        oap = stack2.enter_context(tc.tile_pool(name="oap", bufs=1, space="PSUM"))
        pjp = stack2.enter_context(tc.tile_pool(name="pjp", bufs=2, space="PSUM"))

        # ---- projection / drain unit helpers ----
        def qproj_half(half, m, qc, ps, lo):
            # 3 contraction tiles; lo selects kt 0-2 (start) or 3-5 (stop)
            rows = 128 if m == 0 else 64
            msl = ds(m * 128, rows)
            for kt in range(3 * lo, 3 * lo + 3):
                nc.tensor.matmul(
                    ps[:rows, :],
                    sb_wq[:, kt, msl],
                    sb_xT[:, half, qc, kt, :],
                    start=(kt == 0),
                    stop=(kt == KT_D - 1),
                )
            if lo == 1:
                nc.vector.tensor_scalar_add(
                    out=sb_qT[:rows, m, ds(half * 1024 + qc * 512, 512)],
                    in0=ps[:rows, :],
                    scalar1=sb_bq[:rows, m : m + 1],
                )

        def kproj_half(m, c, ps, lo):
            rows = 128 if m == 0 else 64
            msl = ds(m * 128, rows)
            nkt_c = min(4 * c + 4, nk_t) - 4 * c
            for kt in range(3 * lo, 3 * lo + 3):
                nc.tensor.matmul(
                    ps[:rows, 0 : nkt_c * 128],
                    sb_wk[:, kt, msl],
                    sb_xkT[:, ds(4 * c, nkt_c), kt, :],
                    start=(kt == 0),
                    stop=(kt == KT_D - 1),
                )
            if lo == 1:
                nc.vector.tensor_scalar_add(
                    out=sb_kT[:rows, m, ds(c * 512, nkt_c * 128)],
                    in0=ps[:rows, 0 : nkt_c * 128],
                    scalar1=sb_bk[:rows, m : m + 1],
                )

        def vproj(kt2):
            ps = pjp.tile([128, 512], F32, tag="ps")
            for kt in range(KT_D):
                nc.tensor.matmul(
                    ps[:, 0:192],
                    sb_xkT[:, kt2, kt, :],
                    sb_wv[:, kt, :],
                    start=(kt == 0),
                    stop=(kt == KT_D - 1),
                )
            nc.vector.tensor_copy(
                out=sb_v[:, :, kt2, 0:64],
                in_=ps[:, 0:192].rearrange("p (h d) -> p h d", h=NH),
            )

        def oproj_chunk(qt, og, j, eoff, ech, pool, cast):
            ps = pool.tile([128, 512], F32, tag="ps")
            nc.tensor.matmul(
                ps[:, :ech],
                sb_onT[:, 0, ts(qt, 128)],
                sb_wo[:, 0, ds(eoff, ech)],
                start=True,
                stop=False,
            )
            nc.tensor.matmul(
                ps[:, :ech],
                sb_onT[0:64, 1, ts(qt, 128)],
                sb_wo[0:64, 1, ds(eoff, ech)],
                start=False,
                stop=True,
            )
            if cast == "scalar":
                nc.scalar.copy(out=og[:, j, ds(eoff, ech)], in_=ps[:, :ech])
            else:
                nc.vector.tensor_copy(out=og[:, j, ds(eoff, ech)], in_=ps[:, :ech])

        # ---- attention step helpers ----
        def head_rows(head):
            qrow = 64 if head == 1 else 0
            slot = 1 if head == 2 else 0
            return qrow, slot

        def qk(head, half, kt2, sT):
            qrow, slot = head_rows(head)
            for qc in range(2):
                nc.tensor.matmul(
                    sT[:, ts(qc, 512)],
                    sb_kT[ds(qrow, 64), slot, ts(kt2, 128)],
                    sb_qT[ds(qrow, 64), slot, ds(half * 1024 + qc * 512, 512)],
                    start=True,
                    stop=True,
                )

        def pv(head, kt2, et, oacc):
            for qc in range(2):
                nc.tensor.matmul(
                    oacc[:, ts(qc, 512)],
                    sb_v[:, head, kt2, :],
                    et[:, ts(qc, 512)],
                    start=(kt2 == 0),
                    stop=(kt2 == nk_t - 1),
                )

        def norm(head, half, oacc):
            # evacuate oacc to SBUF first (frees the single PSUM oacc for
            # the next head's PV with ~1.5us latency), then normalize
            # entirely SBUF-side off the critical path.
            qrow, slot = head_rows(head)
            for ch in range(2):
                csl = ds(ch * 512, 512)
                drow = rowp.tile([1, 512], F32, tag="drow")
                nc.vector.tensor_copy(out=drow, in_=oacc[64:65, csl])
                oraw = rowp.tile([64, 512], F32, tag="oraw", name="oraw")
                nc.vector.tensor_copy(out=oraw, in_=oacc[0:64, csl])
                rrow = rowp.tile([1, 512], F32, tag="rrow")
                nc.vector.reciprocal_approx_fast(out=rrow, in_=drow)
                rb = bcp.tile([64, 512], F32, tag="rb")
                nc.gpsimd.partition_broadcast(rb, rrow)
                nc.vector.tensor_mul(
                    out=sb_onT[ds(qrow, 64), slot, ds(half * 1024 + ch * 512, 512)],
                    in0=oraw,
                    in1=rb,
                )

        # ---- deferred unit schedule: at most one half-unit per step ----
        og_h0 = {}

        def oproj_h0(qt):
            if qt % 2 == 0:
                og_h0[qt // 2] = ogp.tile([128, 2, D], F16, tag="og", name="og")
            og = og_h0[qt // 2]
            j = qt % 2
            oproj_chunk(qt, og, j, 0, 512, pjp, "vector")
            oproj_chunk(qt, og, j, 512, 256, pjp, "vector")
            if qt % 2 == 1:
                nc.sync.dma_start(out=d_out[:, ds(qt - 1, 2), :], in_=og)

        units = {}

        def add_unit(step, th):
            units.setdefault(step, []).append(th)

        kp_ps = {}

        def kproj_u(m, c, lo):
            key = (m, c)
            if lo == 0:
                kp_ps[key] = pjp.tile([128, 512], F32, tag="ps", name="ps")
            kproj_half(m, c, kp_ps[key], lo)

        qp_ps = {}

        def qproj_u(half, m, qc, lo):
            key = (half, m, qc)
            if lo == 0:
                qp_ps[key] = pjp.tile([128, 512], F32, tag="ps", name="ps")
            qproj_half(half, m, qc, qp_ps[key], lo)

        # vproj(kt2) feeds PV(A, h0, kt2) one step later; bunch 0-3 into
        # steps 0-1 to free steps 2-3 for the kproj slot0 chunk-1 halves
        # (QK(A, kt2=4) needs kT cols 512+ at step 4).
        add_unit(0, lambda: vproj(0))
        add_unit(0, lambda: vproj(1))
        add_unit(1, lambda: vproj(2))
        add_unit(1, lambda: vproj(3))
        for kt2 in range(4, nk_t):
            add_unit(kt2, lambda k=kt2: vproj(k))
        s = 2
        for c in range(1, nkc):
            add_unit(s, lambda cc=c: kproj_u(0, cc, 0))
            add_unit(s + 1, lambda cc=c: kproj_u(0, cc, 1))
            s += 2
        # kproj m=1 (head C) as half-units; needed by C-h0 at 2*nk_t
        base = nk_t
        for c in range(nkc):
            add_unit(base + 2 * c, lambda cc=c: kproj_u(1, cc, 0))
            add_unit(base + 2 * c + 1, lambda cc=c: kproj_u(1, cc, 1))
        qb = base + 2 * nkc
        add_unit(qb + 0, lambda: qproj_u(0, 1, 0, 0))
        add_unit(qb + 1, lambda: qproj_u(0, 1, 0, 1))
        add_unit(qb + 2, lambda: qproj_u(0, 1, 1, 0))
        add_unit(qb + 3, lambda: qproj_u(0, 1, 1, 1))
        # during C-h0 (steps 2*nk_t..): q projections for half 1
        b2 = 2 * nk_t
        add_unit(b2 + 0, lambda: qproj_u(1, 0, 0, 0))
        add_unit(b2 + 1, lambda: qproj_u(1, 0, 0, 1))
        add_unit(b2 + 2, lambda: qproj_u(1, 0, 1, 0))
        add_unit(b2 + 3, lambda: qproj_u(1, 0, 1, 1))
        add_unit(b2 + 4, lambda: qproj_u(1, 1, 0, 0))
        add_unit(b2 + 5, lambda: qproj_u(1, 1, 0, 1))
        add_unit(b2 + 6, lambda: qproj_u(1, 1, 1, 0))
        add_unit(b2 + 7, lambda: qproj_u(1, 1, 1, 1))
        # oproj for half 0 (q tiles 0-7), one tile every other step during
        # half-1 attention (offset +3 so the C-h0 normalize has landed)
        b3 = 3 * nk_t
        for i in range(8):
            add_unit(min(b3 + 3 + 2 * i, 6 * nk_t - 1), lambda q=i: oproj_h0(q))

        # ---- lead-in: minimum to start (A, h0) ----
        kproj_u(0, 0, 0)
        kproj_u(0, 0, 1)
        qproj_u(0, 0, 0, 0)
        qproj_u(0, 0, 0, 1)
        qproj_u(0, 0, 1, 0)
        qproj_u(0, 0, 1, 1)

        # ---- 48-step attention pipeline ----
        order = [(0, 0), (0, 1), (0, 2), (1, 0), (1, 1), (1, 2)]
        prev = None
        step = 0
        for half, head in order:
            oacc = oap.tile([65, 1024], F32, tag="oacc")
            for kt2 in range(nk_t):
                sT = stp.tile([128, 1024], F32, tag="sT")
                qk(head, half, kt2, sT)
                et = etp.tile([128, 1024], F16, tag="et")
                nc.scalar.activation(et, sT, AFT.Exp, bias=sb_soff[:, 0:1])
                if prev is not None:
                    ph, phalf, pkt2, pet, poacc = prev
                    pv(ph, pkt2, pet, poacc)
                    if pkt2 == nk_t - 1:
                        norm(ph, phalf, poacc)
                for th in units.pop(step, []):
                    th()
                prev = (head, half, kt2, et, oacc)
                step += 1
        ph, phalf, pkt2, pet, poacc = prev
        pv(ph, pkt2, pet, poacc)
        norm(ph, phalf, poacc)
        for s in sorted(units):
            for th in units.pop(s):
                th()

        # keep the PE HAM-warm through the final normalize window: a few
        # throwaway matmuls into a scratch bank (result never read)
        if False:
            warm = pjp.tile([128, 512], F32, tag="ps", name="warm")
            for _ in range(5):
                nc.tensor.matmul(
                    warm, sb_wq[:, 0, 0:128], sb_qT[:, 0, 0:512], start=True, stop=True
                )

        # close attention PSUM pools; tail gets its own 4-buffer pool
        stack2.close()
        tailp = ctx.enter_context(tc.tile_pool(name="tailp", bufs=4, space="PSUM"))

        # ---- tail: oproj for half 1 (q tiles 8-15) ----
        for i in range(4):
            og = ogp.tile([128, 2, D], F16, tag="og")
            for j in range(2):
                qt = 8 + 2 * i + j
                oproj_chunk(qt, og, j, 0, 512, tailp, "scalar")
                oproj_chunk(qt, og, j, 512, 256, tailp, "vector")
            eng = nc.sync if i % 2 == 0 else nc.scalar
            eng.dma_start(out=d_out[:, ds(8 + 2 * i, 2), :], in_=og)

    nc.compile()
    return nc


def kernel(
    hidden_states,
    complexity_scores,
    attention_mask,
    Wq,
    bq,
    Wk,
    bk,
    Wv,
    bv,
    Wo,
    bo,
    emb_table,
    comp_scaling,
):
    global LAST_EXEC_TIME_NS, LAST_RESULTS
    hs = np.asarray(hidden_states, np.float32)
    cs = np.asarray(complexity_scores).astype(np.int64)
    am = np.asarray(attention_mask)
    Wq = np.asarray(Wq, np.float32)
    bq = np.asarray(bq, np.float32)
    Wk = np.asarray(Wk, np.float32)
    bk = np.asarray(bk, np.float32)
    Wv = np.asarray(Wv, np.float32)
    bv = np.asarray(bv, np.float32)
    Wo = np.asarray(Wo, np.float32)
    bo = np.asarray(bo, np.float32)
    emb_table = np.asarray(emb_table, np.float32)
    comp_scaling = np.asarray(comp_scaling, np.float32)

    # per-head score scale (identical across batch: mean over batch of embs)
    embs = emb_table[cs]  # (B, H)
    scal = comp_scaling * embs.mean(axis=0)  # (H,)
    c = (scal / math.sqrt(HD)).astype(np.float32)

    # gather unmasked keys per batch; pad to a common multiple of 128
    idx = [np.nonzero(am[b] != 0)[0] for b in range(B)]
    n_max = max(1, max(len(i) for i in idx))
    nk_t = max(2, (n_max + 127) // 128)
    n_k = nk_t * 128
    nkc = (n_k + 511) // 512
    nkp = nkc * 512

    xT = []
    xkT = []
    vcol = []
    for b in range(B):
        t = hs[b].T.astype(np.float16)  # (768, 2048)
        # (128, half, qc, kt, 512)
        xT.append(
            np.ascontiguousarray(
                t.reshape(KT_D, 128, 2, 2, 512).transpose(1, 2, 3, 0, 4)
            )
        )
        tk = np.zeros((D, nk_t * 128), np.float16)
        tk[:, : len(idx[b])] = hs[b][idx[b]].T
        # (128, nk_t, KT_D, 128): key-tile-major, contiguous per partition
        xkT.append(
            np.ascontiguousarray(
                tk.reshape(KT_D, 128, nk_t, 128).transpose(1, 2, 0, 3)
            )
        )
        v = np.zeros((nk_t * 128,), np.float16)
        v[: len(idx[b])] = 1.0
        vcol.append(np.ascontiguousarray(v.reshape(nk_t, 128).T))

    WqT = Wq.T  # (d_in, e_out)
    WkT = Wk.T
    WvT = Wv.T
    WoT = np.ascontiguousarray(Wo.T)  # rows = attended feature d

    def pack_w(w192):  # (768, 192) -> (128, KT_D, 192)
        return np.ascontiguousarray(
            w192.astype(np.float16).reshape(KT_D, 128, 192).transpose(1, 0, 2)
        )

    def pack_bias(vec):  # (192,) -> (128, 2)
        out = np.zeros((128, 2), np.float32)
        out[:, 0] = vec[:128]
        out[:64, 1] = vec[128:]
        return out

    in_maps = []
    for core in range(8):
        b = core // 4
        heads = [3 * (core % 4) + j for j in range(NH)]
        cols = np.concatenate([np.arange(h * HD, (h + 1) * HD) for h in heads])
        cscale = np.repeat(c[heads], HD)  # (192,)
        wq_c = pack_w(WqT[:, cols] * cscale[None, :])
        bq_c = bq[cols] * cscale
        wk_c = pack_w(WkT[:, cols])
        bk_c = bk[cols]
        wv_c = pack_w(WvT[:, cols])
        wo_c = np.zeros((128, 2, D), np.float16)
        wo_c[:, 0, :] = WoT[cols[:128], :]
        wo_c[:64, 1, :] = WoT[cols[128:], :]
        in_maps.append(
            {
                "xT": xT[b],
                "xkT": xkT[b],
                "wq": wq_c,
                "wk": wk_c,
                "wv": wv_c,
                "wo": np.ascontiguousarray(wo_c),
                "bq": pack_bias(bq_c),
                "bk": pack_bias(bk_c),
                "vcol": vcol[b],
            }
        )

    nc = build_nc(nk_t)
    trace = os.environ.get("KERNEL_TRACE", "0") == "1"
    res = run_bass_kernel_spmd(nc, in_maps, core_ids=list(range(8)), trace=trace)
    LAST_EXEC_TIME_NS = res.exec_time_ns
    LAST_RESULTS = res

    bo_eff = (bo + Wo @ bv).astype(np.float64)
    out = np.empty((B, S, D), np.float32)
    for b in range(B):
        acc = np.zeros((S, D), np.float64)
        for g in range(4):
            p = res.results[4 * b + g]["out"]  # (128, 16, D) fp16
            acc += p.astype(np.float64).transpose(1, 0, 2).reshape(S, D)
        out[b] = (acc + bo_eff[None, :]).astype(np.float32)
    return out
